# revision 1
# baseline (speedup 1.0000x reference)
"""Trainium2 Bass kernel for nn_Attention_90125593739547.

Full-input contract: kernel(**inputs) takes the unsharded numpy inputs and
returns the full [S, B, D] output. Internally:
  - 8 NeuronCores, core c handles batch b = c // 4 and 4 heads (c % 4).
  - Per-core program (all matmuls fp32r, transposed-scores attention):
      xT = x.T done on HOST during sharding; DMA'd   [1024d, 2048t]
      qT = Wq.T @ xT + bq; kT = Wk.T @ xT + bk      [256hd, 2048]
      V~ = x @ [Wv | 0] + [bv | 1]                  [2048t, 4*65]  (ones col)
      per head pair, per q-half:
        St = kT_h.T-slice @ qT_h-slice (row-packed pairs)  [128t, 2*1024]
        pT = exp(SCALE * St)            (ScalarE, fp32r out)
        pv[65, 1024] += V~_h.T @ pT_h   (row 64 accumulates softmax denom)
        OT = pv[0:64] * recip(bcast(denom))   (K=1 ones matmul broadcast)
      y_partial = OT.T @ Wp_slice + bp_share        [2048, 1024]
  - Host sums the 4 per-head-group partials per batch (tensor-parallel reduce).
"""
import sys
sys.path.insert(0, '/opt/trn_rl_repo')
import numpy as np
from contextlib import ExitStack

S, B, D = 2048, 2, 1024
H, HD = 16, 64
SCALE = 1.0 / (HD ** 0.5)
P = 128
N_CORES = 8
CORES_PER_B = 4
NH = H // CORES_PER_B          # heads per core = 4
HDL = NH * HD                  # local head width = 256
SQ = S                         # q span per core (full sequence)
QH = 1024                      # q processed per attention stripe

_cache = {}


def _build():
    import concourse.bacc as bacc
    import concourse.mybir as mybir
    from concourse import tile

    F32 = mybir.dt.float32
    F32R = mybir.dt.float32r
    AF = mybir.ActivationFunctionType

    n_d, n_t, n_m = D // P, S // P, HDL // P
    n_qh = SQ // QH
    NV = NH * 65

    nc = bacc.Bacc("TRN2", target_bir_lowering=False, debug=False,
                   num_devices=N_CORES)

    x = nc.dram_tensor("x", [D, S], F32R, kind="ExternalInput")  # pre-transposed on host
    wq = nc.dram_tensor("wq", [D, HDL], F32R, kind="ExternalInput")
    wk = nc.dram_tensor("wk", [D, HDL], F32R, kind="ExternalInput")
    wv = nc.dram_tensor("wv", [D, NV], F32R, kind="ExternalInput")
    bq = nc.dram_tensor("bq", [1, HDL], F32R, kind="ExternalInput")
    bk = nc.dram_tensor("bk", [1, HDL], F32R, kind="ExternalInput")
    bv = nc.dram_tensor("bv", [1, NV], F32R, kind="ExternalInput")
    wp = nc.dram_tensor("wp", [HDL, D], F32R, kind="ExternalInput")
    bp = nc.dram_tensor("bp", [1, D], F32R, kind="ExternalInput")
    ones_d = nc.dram_tensor("ones", [1, 512], F32R, kind="ExternalInput")
    y = nc.dram_tensor("y", [SQ, D], F32, kind="ExternalOutput")

    with tile.TileContext(nc) as tc, ExitStack() as ctx:
        const = ctx.enter_context(tc.tile_pool(name="const", bufs=1))
        ones_row_t = const.tile([1, 512], F32R)
        nc.sync.dma_start(ones_row_t[:], ones_d[:, :])
        ones_row = ones_row_t[:]

        kv_pool = ctx.enter_context(tc.tile_pool(name="kv", bufs=1))
        qT = [kv_pool.tile([P, SQ], F32R, tag=f"qT{m}", name=f"qT{m}") for m in range(n_m)]
        kT = [kv_pool.tile([P, S], F32R, tag=f"kT{m}", name=f"kT{m}") for m in range(n_m)]
        Vt = [kv_pool.tile([P, NV], F32R, tag=f"V{t}", name=f"V{t}") for t in range(n_t)]
        OT = [kv_pool.tile([P, SQ], F32R, tag=f"OT{m}", name=f"OT{m}") for m in range(n_m)]

        proj_pool = ctx.enter_context(tc.tile_pool(name="proj", bufs=1))
        wp_sb = [proj_pool.tile([P, D], F32R, tag=f"wp{m}", name=f"wp{m}")
                 for m in range(n_m)]
        for m in range(n_m):
            nc.sync.dma_start(wp_sb[m][:], wp[m * P:(m + 1) * P, :])
        ystream = ctx.enter_context(tc.tile_pool(name="ystream", bufs=4))

        bias_pool = ctx.enter_context(tc.tile_pool(name="bias", bufs=1))
        bq_t = bias_pool.tile([1, HDL], F32R, tag="bq", name="bq")
        bk_t = bias_pool.tile([1, HDL], F32R, tag="bk", name="bk")
        bv_t = bias_pool.tile([1, NV], F32R, tag="bv", name="bv")
        bp_t = bias_pool.tile([1, D], F32R, tag="bp", name="bp")
        nc.sync.dma_start(bq_t[:], bq[:, :])
        nc.sync.dma_start(bk_t[:], bk[:, :])
        nc.sync.dma_start(bv_t[:], bv[:, :])
        nc.sync.dma_start(bp_t[:], bp[:, :])

        # ---- Phases A+B: transpose + QKV (xT/weights freed afterwards) ----
        with tc.tile_pool(name="xw", bufs=1) as xw_pool, \
             tc.tile_pool(name="psumAB", bufs=1, space="PSUM") as psum:
            xT = [xw_pool.tile([P, S], F32R, tag=f"xT{d}", name=f"xT{d}") for d in range(n_d)]
            wq_sb = [xw_pool.tile([P, HDL], F32R, tag=f"wq{d}", name=f"wq{d}") for d in range(n_d)]
            wk_sb = [xw_pool.tile([P, HDL], F32R, tag=f"wk{d}", name=f"wk{d}") for d in range(n_d)]
            wv_sb = [xw_pool.tile([P, NV], F32R, tag=f"wv{d}", name=f"wv{d}") for d in range(n_d)]
            # A: xT arrives pre-transposed from the host; straight DMA loads.
            # First halves land first so V/kq chains start after ~4MB.
            for half in (0, 1):
                for d in range(n_d):
                    nc.sync.dma_start(xT[d][:, half * 1024:(half + 1) * 1024],
                                      x[d * P:(d + 1) * P, half * 1024:(half + 1) * 1024])
                if half == 0:
                    for dd in range(n_d):
                        nc.sync.dma_start(wv_sb[dd][:], wv[dd * P:(dd + 1) * P, :])
                        nc.sync.dma_start(wk_sb[dd][:], wk[dd * P:(dd + 1) * P, :])
                        nc.sync.dma_start(wq_sb[dd][:], wq[dd * P:(dd + 1) * P, :])

            # B: V~ = x @ [Wv|0] + [bv|1]  (first: attention needs all of V)
            for tt in range(n_t):
                ps = psum.tile([P, NV], F32, tag="qkv2", name="qkv2", bufs=2)
                for lo in range(0, NV, 512):
                    w = min(512, NV - lo)
                    for dt in range(n_d):
                        nc.tensor.matmul(ps[:, lo:lo + w],
                                         xT[dt][:, tt * P:(tt + 1) * P],
                                         wv_sb[dt][:, lo:lo + w],
                                         start=(dt == 0), stop=False)
                    nc.tensor.matmul(ps[:, lo:lo + w], ones_row[0:1, 0:P],
                                     bv_t[0:1, lo:lo + w], start=False, stop=True)
                nc.vector.tensor_copy(Vt[tt][:], ps[:])

            # B: qT, kT (+bias via K=1 ones matmul), 512-wide chains
            kqi = 0
            for m in range(n_m):
                for dst, wsb, bias in ((kT, wk_sb, bk_t), (qT, wq_sb, bq_t)):
                    for lo in range(0, S, 512):
                        ps = psum.tile([P, 512], F32, tag="qkv", name="qkv", bufs=3)
                        for dt in range(n_d):
                            nc.tensor.matmul(
                                ps[:], wsb[dt][:, m * P:(m + 1) * P],
                                xT[dt][:, lo:lo + 512],
                                start=(dt == 0), stop=False)
                        nc.tensor.matmul(ps[:], bias[0:1, m * P:(m + 1) * P],
                                         ones_row[0:1, 0:512],
                                         start=False, stop=True)
                        if kqi % 2 == 0:
                            nc.vector.tensor_copy(dst[m][:, lo:lo + 512], ps[:])
                        else:
                            nc.scalar.copy(dst[m][:, lo:lo + 512], ps[:])
                        kqi += 1

        # ---- Phase C: attention ----
        # stripes: (q-block of 512) major, head-pair minor -> projection can
        # start on finished q-blocks while later stripes still run.
        QB = 512
        with tc.tile_pool(name="attn", bufs=2) as attn_pool, \
             tc.tile_pool(name="psumC", bufs=1, space="PSUM") as psum:
            for qb in range(SQ // QB):
                qlo = qb * QB
                for m in range(n_m):
                    pvA = psum.tile([65, QB], F32, tag="pvA", name="pvA", bufs=1)
                    pvB = psum.tile([65, QB], F32, tag="pvB", name="pvB", bufs=1)
                    for tt in range(n_t):
                        sc = psum.tile([P, 2 * QB], F32, tag="sc", name="sc", bufs=2)
                        for half, plo in ((0, 0), (1, 64)):
                            nc.tensor.matmul(
                                sc[:, half * QB: half * QB + QB],
                                kT[m][plo:plo + 64, tt * P:(tt + 1) * P],
                                qT[m][plo:plo + 64, qlo: qlo + QB],
                                start=True, stop=True)
                        pT = attn_pool.tile([P, 2 * QB], F32R, tag="pT", name="pT",
                                            bufs=4)
                        nc.scalar.activation(pT[:], sc[:], AF.Exp, scale=SCALE)
                        for half, pv in ((0, pvA), (1, pvB)):
                            h = 2 * m + half
                            nc.tensor.matmul(
                                pv[:], Vt[tt][:, h * 65:(h + 1) * 65],
                                pT[:, half * QB: half * QB + QB],
                                start=(tt == 0), stop=(tt == n_t - 1))
                    for half, pv in ((0, pvA), (1, pvB)):
                        plo = half * 64
                        den = attn_pool.tile([1, QB], F32, tag="den", name="den", bufs=3)
                        nc.vector.tensor_copy(den[:], pv[64:65, :])
                        ov = attn_pool.tile([64, QB], F32, tag="ov", name="ov", bufs=3)
                        nc.vector.tensor_copy(ov[:], pv[0:64, :])
                        dnb = attn_pool.tile([64, QB], F32, tag="dnb", name="dnb")
                        nc.gpsimd.partition_broadcast(dnb[:], den[0:1, :])
                        rcb = attn_pool.tile([64, QB], F32, tag="rcb", name="rcb")
                        nc.vector.reciprocal_approx_fast(rcb[:], dnb[:])
                        nc.vector.tensor_tensor(
                            OT[m][plo:plo + 64, qlo:qlo + QB],
                            ov[:], rcb[:], op=mybir.AluOpType.mult)

            # ---- Phase D: projection ----
            for qt in range(SQ // P):
                for nn in range(0, D, 512):
                    ps = psum.tile([P, 512], F32, tag="y", name="y", bufs=2)
                    for m in range(n_m):
                        nc.tensor.matmul(ps[:], OT[m][:, qt * P:(qt + 1) * P],
                                         wp_sb[m][:, nn:nn + 512],
                                         start=(m == 0), stop=False)
                    nc.tensor.matmul(ps[:], ones_row[0:1, 0:P], bp_t[0:1, nn:nn + 512],
                                     start=False, stop=True)
                    yt = ystream.tile([P, 512], F32, tag="y_out", name="y_out")
                    nc.vector.tensor_copy(yt[:], ps[:])
                    nc.sync.dma_start(y[qt * P:(qt + 1) * P, nn:nn + 512], yt[:])

    nc.compile()
    return nc


def _get_nc():
    if "nc" not in _cache:
        _cache["nc"] = _build()
    return _cache["nc"]


def make_in_maps(inputs, Wkv, bkv, Wq, bq, Wp, bp):
    """Host-side sharding: per-core input dicts."""
    inputs = np.asarray(inputs, dtype=np.float32)
    Wkv = np.asarray(Wkv, dtype=np.float32)
    bkv = np.asarray(bkv, dtype=np.float32)
    Wq = np.asarray(Wq, dtype=np.float32)
    bq = np.asarray(bq, dtype=np.float32)
    Wp = np.asarray(Wp, dtype=np.float32)
    bp = np.asarray(bp, dtype=np.float32)

    ones_np = np.ones((1, 512), dtype=np.float32)
    bp_np = bp.reshape(1, D)
    zeros_bp = np.zeros((1, D), dtype=np.float32)

    in_maps = []
    for c in range(N_CORES):
        b = c // CORES_PER_B
        g = c % CORES_PER_B
        hsl = slice(g * HDL, (g + 1) * HDL)
        x_b = np.ascontiguousarray(inputs[:, b, :].T)
        wq_c = np.ascontiguousarray(Wq[:, hsl])
        bq_c = np.ascontiguousarray(bq[hsl]).reshape(1, HDL)
        wk_c = np.ascontiguousarray(Wkv[:, hsl])
        bk_c = np.ascontiguousarray(bkv[hsl]).reshape(1, HDL)
        wv_full = Wkv[:, H * HD + g * HDL: H * HD + (g + 1) * HDL]
        bv_full = bkv[H * HD + g * HDL: H * HD + (g + 1) * HDL]
        wv_c = np.zeros((D, NH * 65), dtype=np.float32)
        bv_c = np.zeros((1, NH * 65), dtype=np.float32)
        for h in range(NH):
            wv_c[:, h * 65:h * 65 + 64] = wv_full[:, h * 64:(h + 1) * 64]
            bv_c[0, h * 65:h * 65 + 64] = bv_full[h * 64:(h + 1) * 64]
            bv_c[0, h * 65 + 64] = 1.0
        wp_c = np.ascontiguousarray(Wp[hsl, :])
        in_maps.append(dict(
            x=x_b, wq=wq_c, wk=wk_c, wv=wv_c, bq=bq_c, bk=bk_c, bv=bv_c,
            wp=wp_c, bp=(bp_np if g == 0 else zeros_bp),
            ones=ones_np))
    return in_maps


def combine_outputs(results):
    """Host-side unshard: sum head-group partials per batch."""
    out = np.zeros((S, B, D), dtype=np.float32)
    for b in range(B):
        acc = results[b * CORES_PER_B]["y"].copy()
        for g in range(1, CORES_PER_B):
            acc += results[b * CORES_PER_B + g]["y"]
        out[:, b, :] = acc
    return out


def kernel(inputs, Wkv, bkv, Wq, bq, Wp, bp):
    from concourse.bass_utils import run_bass_kernel_spmd
    nc = _get_nc()
    in_maps = make_in_maps(inputs, Wkv, bkv, Wq, bq, Wp, bp)
    res = run_bass_kernel_spmd(nc, in_maps, list(range(N_CORES)))
    return combine_outputs(res.results)



# revision 24
# speedup vs baseline: 1.1728x; 1.1728x over previous
"""Trainium2 Bass kernel for nn_Attention_90125593739547.

Full-input contract: kernel(**inputs) takes the unsharded numpy inputs and
returns the full [S, B, D] output. Internally:
  - 8 NeuronCores, core c handles batch b = c // 4 and 4 heads (c % 4).
  - Softmax algebra moves biases off the TensorE: the k-bias shifts all
    logits of a softmax row equally (dropped), the v-bias and output bias
    are linear post-terms (added on host), only the q-bias survives (one
    per-partition DVE add at evacuation).
  - Per-core program (bf16 matmuls, optional fp8e4m3 DoubleRow attn@V):
      kT/qT = W.T @ x          [128 (2 heads x 64), 2048] bf16, N=2048 chains
      V2    = x @ Wv stored per t-tile-pair in a DoubleRow-folded layout
              [128, 4h * 2j * 68]; column 64 of each 68-block is memset to 1
              so the PV matmul also accumulates the softmax denominator.
      per head, per q-chunk of 1024:
        sc  = kT_h.T @ qT_h per t-pair          [128, 2 * 1024] PSUM
        pT  = exp(SCALE * sc + C)               one ScalarE op; C keeps the
                                                fp8 values in normal range
                                                and cancels in the ratio
        pv += V2_pair.T @ pT  (DoubleRow K=256) [65, 1024]; row 64 = sum p
        OT  = pv[0:64] * recip(pv[64])          normalization deferred off
                                                the PSUM drain path
      y_partial = OT.T @ Wp                     [2048, 1024] bf16 out
  - Host sums the 4 per-head-group partials per batch and adds bv@Wp + bp.
  - V~ production, the m=1 K/Q chains and the qc0 projection are interleaved
    into the PE slots of the ScalarE-bound attention stream.
"""
import sys
sys.path.insert(0, '/opt/trn_rl_repo')
import numpy as np
from contextlib import ExitStack

S, B, D = 2048, 2, 1024
H, HD = 16, 64
SCALE = 1.0 / (HD ** 0.5)
P = 128
N_CORES = 8
CORES_PER_B = 4
NH = H // CORES_PER_B          # heads per core = 4
HDL = NH * HD                  # local head width = 256
CSHIFT = 2.75                  # exp shift: keeps p' in fp8e4m3 normal range
JVW = 80                       # V2 j-block stride: DoubleRow needs step%16==0
HVW = 2 * JVW                  # per-head V2 stride = 160
NV = NH * HVW                  # V2 row width = 640
QC = 512                       # q-chunk per attention stripe

USE_FP8_PV = True
DEBUG_DUMP = False

_cache = {}


def _build(fp8=USE_FP8_PV, reps=1):
    import concourse.bacc as bacc
    import concourse.mybir as mybir
    from concourse import tile

    nc = bacc.Bacc("TRN2", target_bir_lowering=False, debug=False,
                   num_devices=N_CORES)

    F32 = mybir.dt.float32
    BF16 = mybir.dt.bfloat16
    x = nc.dram_tensor("x", [D, S], BF16, kind="ExternalInput")
    wkqv = nc.dram_tensor("wkqv", [D, 3 * HDL], BF16, kind="ExternalInput")
    bq = nc.dram_tensor("bq", [P, 2], F32, kind="ExternalInput")
    wp = nc.dram_tensor("wp", [HDL, D], BF16, kind="ExternalInput")
    y = nc.dram_tensor("y", [S, D], BF16, kind="ExternalOutput")
    dbg = None
    if DEBUG_DUMP:
        dbg = dict(
            csh=nc.dram_tensor("dbg_csh", [P, 1], F32, kind="ExternalOutput"),
            v2=nc.dram_tensor("dbg_v2", [P, NH * HVW], mybir.dt.float8e4 if fp8 else BF16,
                              kind="ExternalOutput"),
            ot=nc.dram_tensor("dbg_ot", [P, S], BF16, kind="ExternalOutput"),
            kt=nc.dram_tensor("dbg_kt", [P, S], BF16, kind="ExternalOutput"),
            pv=nc.dram_tensor("dbg_pv", [65, QC], F32, kind="ExternalOutput"),
            pt=nc.dram_tensor("dbg_pt", [P, 2 * QC], BF16, kind="ExternalOutput"))

    with tile.TileContext(nc) as tc, ExitStack() as octx:
        if reps > 1:
            octx.enter_context(tc.For_i(0, reps))
        with ExitStack() as ctx:
            _body(nc, tc, ctx, mybir, fp8, x, wkqv, bq, wp, y, dbg)
    nc.compile()
    return nc


def _body(nc, tc, ctx, mybir, fp8, x, wkqv, bq, wp, y, dbg=None):
    F32 = mybir.dt.float32
    BF16 = mybir.dt.bfloat16
    P_DT = mybir.dt.float8e4 if fp8 else BF16
    AF = mybir.ActivationFunctionType
    n_d, n_t = D // P, S // P
    n_qc, n_r = S // QC, n_t // 2

    # ---------------- persistent SBUF ----------------
    const = ctx.enter_context(tc.tile_pool(name="const", bufs=1))
    xb = [const.tile([P, S], BF16, tag=f"x{d}", name=f"x{d}") for d in range(n_d)]
    wkqv_sb = [const.tile([P, 3 * HDL], BF16, tag=f"wkqv{d}", name=f"wkqv{d}")
               for d in range(n_d)]
    wk_sb = [t[:, 0:HDL] for t in wkqv_sb]
    wq_sb = [t[:, HDL:2 * HDL] for t in wkqv_sb]
    wv_sb = [t[:, 2 * HDL:3 * HDL] for t in wkqv_sb]
    bq_sb = const.tile([P, 2], F32, tag="bq", name="bq")
    wp_sb = [const.tile([P, D], BF16, tag=f"wp{m}", name=f"wp{m}") for m in range(2)]
    kT = [const.tile([P, S], BF16, tag=f"kT{m}", name=f"kT{m}") for m in range(2)]
    qT = [const.tile([P, S], BF16, tag=f"qT{m}", name=f"qT{m}") for m in range(2)]
    V2 = [const.tile([P, NV], P_DT, tag=f"V2{r}", name=f"V2{r}") for r in range(n_r)]
    OT = [const.tile([P, S], BF16, tag=f"OT{m}", name=f"OT{m}") for m in range(2)]
    work = ctx.enter_context(tc.tile_pool(name="work", bufs=1))
    ystream = ctx.enter_context(tc.tile_pool(name="ystream", bufs=4))
    rc_pool = ctx.enter_context(tc.tile_pool(name="rc", bufs=1))

    # ---------------- DMA in ----------------
    # The HWDGE costs ~630ns per DMA instruction, the movers run ~360GB/s:
    # keep transfers >= 256KB and the instruction count low.  Weights for
    # k/q/v travel as one 192KB transfer per d-block; x as half-tiles so the
    # kq chains and V~ tiles can chase the stream.
    dq = [nc.sync, nc.scalar]
    qi = 0

    def dma(out, in_):
        nonlocal qi
        dq[qi % 2].dma_start(out, in_)
        qi += 1

    for d in range(n_d):
        dma(wkqv_sb[d][:], wkqv[d * P:(d + 1) * P, :])
    for d in range(n_d):
        dma(xb[d][:, 0:1024], x[d * P:(d + 1) * P, 0:1024])
    dma(bq_sb[:], bq[:, :])
    for d in range(n_d):
        dma(xb[d][:, 1024:S], x[d * P:(d + 1) * P, 1024:S])
    for m in range(2):
        dma(wp_sb[m][:], wp[m * P:(m + 1) * P, :])

    # ones columns of V2 (softmax denominator rows), written once
    for r in range(n_r):
        col = V2[r][:, :].rearrange("p (h c) -> p h c", h=NH)
        for j in range(2):
            nc.vector.memset(col[:, :, j * JVW + 64:j * JVW + 65], 1.0)
    csh = const.tile([P, 1], F32, tag="csh", name="csh")
    nc.gpsimd.memset(csh[:], CSHIFT)

    # ---------------- PSUM pools (16KB/partition = 8 banks) ----------------
    sc_pool = ctx.enter_context(tc.tile_pool(name="sc", bufs=1, space="PSUM"))    # 2x2 banks
    pv_pool = ctx.enter_context(tc.tile_pool(name="pv", bufs=1, space="PSUM"))    # 1 bank
    chain = ctx.enter_context(tc.tile_pool(name="chain", bufs=1, space="PSUM"))   # 3 banks

    def kq_chunk(dst, wsb, m, lo, is_q):
        ps = chain.tile([P, 512], F32, tag="kq", name="kq", bufs=2)
        for d in range(n_d):
            nc.tensor.matmul(ps[:], wsb[d][:, m * P:(m + 1) * P],
                             xb[d][:, lo:lo + 512],
                             start=(d == 0), stop=(d == n_d - 1))
        if is_q:
            nc.vector.tensor_scalar(dst[m][:, lo:lo + 512], ps[:],
                                    bq_sb[:, m:m + 1], None,
                                    op0=mybir.AluOpType.add)
        else:
            nc.vector.tensor_copy(dst[m][:, lo:lo + 512], ps[:])

    def v_tile(tt):
        """V~ for t-tile tt -> folded slot j=tt%2 of pair tile V2[tt//2]."""
        vp = chain.tile([P, 512], F32, tag="kq", name="vp", bufs=2)
        for d in range(n_d):
            nc.tensor.matmul(vp[:, 0:HDL], xb[d][:, tt * P:(tt + 1) * P],
                             wv_sb[d][:], start=(d == 0), stop=(d == n_d - 1))
        j = tt % 2
        dst = (V2[tt // 2][:, :]
               .rearrange("p (h c) -> p h c", h=NH)[:, :, j * JVW:j * JVW + 64])
        nc.vector.tensor_copy(
            dst, vp[:, 0:HDL].rearrange("p (h c) -> p h c", h=NH))

    # ---- head phase: only the first k/q chunks.  Later k chunks and all of
    # V~ chase the x column-chunk DMAs inside the first attention stripe.
    kq_chunk(kT, wk_sb, 0, 0, False)
    kq_chunk(qT, wq_sb, 0, 0, True)

    # ---------------- attention stripes ----------------
    # Both heads of an m-pair run interleaved in one stripe: each r-step
    # issues two score groups and two exps, so the ScalarE stream stays fed
    # with half the per-stripe boundary cost.
    def attention_pair(m, qc, interleave=None):
        qlo = qc * QC
        pvs = [pv_pool.tile([65, QC], F32, tag=f"pv{half}", name=f"pv{half}")
               for half in range(2)]
        pTs = [[None] * n_r for _ in range(2)]

        def pv_mm(half, r):
            h = 2 * m + half
            lhsT = (V2[r][:, h * HVW:(h + 1) * HVW]
                    .rearrange("p (j c) -> p j c", j=2)[:, :, 0:65])
            if fp8:
                rhs = pTs[half][r][:, :].rearrange("p (j n) -> p j n", j=2)
                nc.tensor.matmul(pvs[half][:], lhsT, rhs,
                                 perf_mode=mybir.MatmulPerfMode.DoubleRow,
                                 start=(r == 0), stop=(r == n_r - 1))
            else:
                for j in range(2):
                    nc.tensor.matmul(pvs[half][:], lhsT[:, j, :],
                                     pTs[half][r][:, j * QC:(j + 1) * QC],
                                     start=(r == 0 and j == 0),
                                     stop=(r == n_r - 1 and j == 1))

        for r in range(n_r):
            for half in range(2):
                plo = half * 64
                sc = sc_pool.tile([P, 2 * QC], F32, tag="sc", name="sc", bufs=2)
                for j in range(2):
                    nc.tensor.matmul(
                        sc[:, j * QC:(j + 1) * QC],
                        kT[m][plo:plo + 64, (2 * r + j) * P:(2 * r + j + 1) * P],
                        qT[m][plo:plo + 64, qlo:qlo + QC],
                        start=True, stop=True)
                pTs[half][r] = work.tile([P, 2 * QC], P_DT, tag="pT", name="pT",
                                         bufs=6)
                nc.scalar.activation(pTs[half][r][:], sc[:], AF.Exp,
                                     bias=csh[:, 0:1], scale=SCALE)
            if r > 0:
                pv_mm(0, r - 1)
                pv_mm(1, r - 1)
            if interleave is not None:
                interleave(r)    # filler PE work, after the critical ops
        pv_mm(0, n_r - 1)
        pv_mm(1, n_r - 1)
        if dbg is not None and m == 0 and qc == 0:
            stg = rc_pool.tile([65, QC], F32, tag="dbgpv", name="dbgpv")
            nc.vector.tensor_copy(stg[:], pvs[0][:])
            nc.sync.dma_start(dbg["pv"][:, :], stg[:])
            stg2 = rc_pool.tile([P, 2 * QC], BF16, tag="dbgpt", name="dbgpt")
            nc.vector.tensor_copy(stg2[:], pTs[0][n_r - 1][:])
            nc.sync.dma_start(dbg["pt"][:, :], stg2[:])
        for half in range(2):
            plo = half * 64
            pv = pvs[half]
            # drain (releases pv).  The denominator row leaves PSUM via
            # tensor_copy first: reciprocal_approx_fast and partition
            # broadcasts mishandle nonzero base partitions on hardware, so
            # every DVE/Pool op below runs at base partition 0 and the final
            # multiply slices matching partition ranges of both operands.
            den = rc_pool.tile([1, QC], F32, tag="den", name="den", bufs=4)
            nc.vector.tensor_copy(den[:], pv[64:65, :])
            nc.vector.tensor_copy(OT[m][plo:plo + 64, qlo:qlo + QC], pv[0:64, :])
            rc1 = rc_pool.tile([1, QC], F32, tag="rc1", name="rc1", bufs=4)
            nc.vector.reciprocal_approx_fast(rc1[:], den[:])
            rcb = rc_pool.tile([P, QC], F32, tag="rcb", name="rcb", bufs=2)
            nc.gpsimd.partition_broadcast(rcb[:], rc1[0:1, :])
            nc.vector.tensor_tensor(OT[m][plo:plo + 64, qlo:qlo + QC],
                                    OT[m][plo:plo + 64, qlo:qlo + QC],
                                    rcb[plo:plo + 64, :],
                                    op=mybir.AluOpType.mult)

    def proj_steps(qc):
        """Projection of q-chunk qc: one 512-wide output block per step."""
        qlo = qc * QC
        for qt in range(qlo // P, (qlo + QC) // P):
            yt = ystream.tile([P, D], BF16, tag="yt", name="yt")
            for nn in range(0, D, 512):
                ps = chain.tile([P, 512], F32, tag="kq", name="proj", bufs=2)
                for m in range(2):
                    nc.tensor.matmul(ps[:], OT[m][:, qt * P:(qt + 1) * P],
                                     wp_sb[m][:, nn:nn + 512],
                                     start=(m == 0), stop=(m == 1))
                nc.vector.tensor_copy(yt[:, nn:nn + 512], ps[:])
                yield
            nc.sync.dma_start(y[qt * P:(qt + 1) * P, :], yt[:])
        while True:
            yield

    # qc-major stripe order.  (qc0,h0) finishes V~ just-in-time for its own
    # pv accumulation; the m=1 K/Q chains are spread over the (qc0,h1/h2)
    # slack; (qc,h2/h3) produce the next qT m0 chunk; (qc,h0) carries the
    # previous chunk's projection; proj(qc3) is the tail.
    projs = [proj_steps(qc) for qc in range(n_qc)]

    def ilv(table):
        def f(r):
            fn = table.get(r)
            if fn is not None:
                fn()
        return f

    def chase(r):
        # x half-tiles land -> k chunks (sc 2c gates on chunk c) + V~ tiles
        if r == 1:
            kq_chunk(kT, wk_sb, 0, 512, False)
        elif r == 3:
            kq_chunk(kT, wk_sb, 0, 1024, False)
        elif r == 5:
            kq_chunk(kT, wk_sb, 0, 1536, False)
        v_tile(2 * r)
        v_tile(2 * r + 1)
        if r == 6:
            kq_chunk(qT, wq_sb, 0, 512, True)

    attention_pair(0, 0, interleave=chase)
    attention_pair(0, 1, interleave=ilv({
        0: lambda: kq_chunk(qT, wq_sb, 0, 1024, True),
        2: lambda: kq_chunk(qT, wq_sb, 1, 0, True),
        4: lambda: kq_chunk(kT, wk_sb, 1, 0, False),
        6: lambda: kq_chunk(kT, wk_sb, 1, 512, False)}))
    attention_pair(0, 2, interleave=ilv({
        0: lambda: kq_chunk(qT, wq_sb, 0, 1536, True),
        2: lambda: kq_chunk(kT, wk_sb, 1, 1024, False),
        4: lambda: kq_chunk(kT, wk_sb, 1, 1536, False)}))
    attention_pair(0, 3, interleave=ilv({
        0: lambda: kq_chunk(qT, wq_sb, 1, 512, True),
        2: lambda: kq_chunk(qT, wq_sb, 1, 1024, True),
        4: lambda: kq_chunk(qT, wq_sb, 1, 1536, True)}))
    projs = [proj_steps(qc) for qc in range(n_qc)]
    attention_pair(1, 0)
    attention_pair(1, 1, interleave=lambda r: next(projs[0]))
    next(projs[0])           # flush the trailing output DMA
    attention_pair(1, 2, interleave=lambda r: next(projs[1]))
    next(projs[1])
    attention_pair(1, 3, interleave=lambda r: next(projs[2]))
    next(projs[2])
    for _ in range(QC // P * (D // 512) + 1):
        next(projs[n_qc - 1])
    if dbg is not None:
        nc.sync.dma_start(dbg["csh"][:, :], csh[:])
        nc.sync.dma_start(dbg["v2"][:, :], V2[0][:])
        nc.sync.dma_start(dbg["ot"][:, :], OT[0][:])
        nc.sync.dma_start(dbg["kt"][:, :], kT[0][:])


def _get_nc(fp8=USE_FP8_PV, reps=1):
    key = (fp8, reps)
    if key not in _cache:
        _cache[key] = _build(fp8=fp8, reps=reps)
    return _cache[key]


def make_in_maps(inputs, Wkv, bkv, Wq, bq, Wp, bp):
    """Host-side sharding: per-core input dicts (bf16)."""
    import ml_dtypes
    BF = ml_dtypes.bfloat16
    inputs = np.asarray(inputs, dtype=np.float32)
    Wkv = np.asarray(Wkv, dtype=np.float32)
    Wq = np.asarray(Wq, dtype=np.float32)
    bq = np.asarray(bq, dtype=np.float32)
    Wp = np.asarray(Wp, dtype=np.float32)

    in_maps = []
    for c in range(N_CORES):
        b = c // CORES_PER_B
        g = c % CORES_PER_B
        hsl = slice(g * HDL, (g + 1) * HDL)
        wkqv = np.concatenate([
            Wkv[:, hsl], Wq[:, hsl],
            Wkv[:, H * HD + g * HDL: H * HD + (g + 1) * HDL]], axis=1)
        in_maps.append(dict(
            x=np.ascontiguousarray(inputs[:, b, :].T).astype(BF),
            wkqv=np.ascontiguousarray(wkqv).astype(BF),
            bq=np.ascontiguousarray(bq[hsl].reshape(2, P).T),
            wp=np.ascontiguousarray(Wp[hsl, :]).astype(BF)))
    return in_maps


def combine_outputs(results):
    """Host-side unshard: sum the head-group partials per batch."""
    out = np.zeros((S, B, D), np.float32)
    for b in range(B):
        acc = results[b * CORES_PER_B]["y"].astype(np.float32)
        for g in range(1, CORES_PER_B):
            acc += results[b * CORES_PER_B + g]["y"].astype(np.float32)
        out[:, b, :] = acc
    return out


def kernel(inputs, Wkv, bkv, Wq, bq, Wp, bp):
    from concourse.bass_utils import run_bass_kernel_spmd
    nc = _get_nc()
    in_maps = make_in_maps(inputs, Wkv, bkv, Wq, bq, Wp, bp)
    res = run_bass_kernel_spmd(nc, in_maps, list(range(N_CORES)))
    out = combine_outputs(res.results)
    # bias terms hoisted off-device: y += bv @ Wp + bp  (softmax weights sum
    # to one, so the v-bias contributes a constant row through Wp)
    bkv64 = np.asarray(bkv, np.float64)
    bias = (bkv64[H * HD:] @ np.asarray(Wp, np.float64)
            + np.asarray(bp, np.float64)).astype(np.float32)
    out += bias[None, None, :]
    return out
